# revision 9
# baseline (speedup 1.0000x reference)
"""Friend-attention pooling kernel for Trainium2 (8 NeuronCores, SPMD).

Problem (hardcoded shapes):
    friend_diff_x [16384, 50, 128] f32, self_x [256, 128] f32,
    friend_diff_src_mask [16384, 50] bool, friend_num_src == 64.
    out[b, f, :] = sum_l softmax_l(X[n] @ s[b])[l] * mask[n, l] * X[n, l, :]
    with n = b*64 + f.

Strategy: data-parallel over users across 8 cores (2048 friend rows / core,
16 blocks of 128 friends = 2 users each). Host pre-packs friend_diff_x into
TWO fp16 copies so BOTH contractions run on TensorE:
  - xt: per-friend transposed [D=128 part, (friend, L) free] -> phase-1
        score matmuls (contract D) via one-hot stationary columns into a
        dense [16, 400] PSUM tile.
  - xn: mask-premultiplied natural layout, friend-pairs stacked
        [(fip,L)->114 part (50 + 14 zero pad + 50), (pair, D) free]
        -> phase-2 pooling matmuls (contract K=114), X-pair stationary,
        block-diagonal softmax-weight columns moving; output accumulates
        TRANSPOSED [D, friend] densely in PSUM (host un-transposes).
Softmax on [128 friends, 50]: ACT exp with fused accum denominator, DVE
reciprocal + tensor_scalar normalize, PE transpose to [L, friend].

DMA plan (the perf-critical part): the previous revision issued every load
on the single qSP HWDGE ring; its FIFO order serialized the xn row-half
transfers, which only reach 8 of 16 SDMA engines each (partitions 0-63 map
to even engines, 64-127 to odd), capping HBM at ~200 GB/s and leaving the
PE idle ~8us/block. Now loads are split across BOTH HWDGE rings:
  - sync (qSP):    xt columns [0:3200], xn rows 0:50 (even engines)
  - scalar (qAct): xt columns [3200:], xn rows 64:114 (odd engines)
All per-partition segments <= 8KB (16KB rows were observed to fan out to
only 6 of 16 engines). xt/xn buffers are 4-deep; the scalar engine issues
its half of block b+3's loads after its exp(b) work.

Raw bass (manual semaphores); PE stream is software-pipelined (phase1 of
block b issues before transpose/phase2 of block b-1).
"""

from contextlib import ExitStack

import numpy as np

import concourse.bass as bass
from concourse import mybir
from concourse.bass_utils import run_bass_kernel_spmd

B = 256          # users
FPER = 64        # friends per user
L = 50           # history length (softmax axis)
D = 128          # embed dim
N = B * FPER     # 16384 friend rows
NCORES = 8
FCORE = N // NCORES      # 2048 friend rows per core
BF = 128                 # friends per block (= 2 users)
NPAIR = BF // 2          # 64 pairs per block
NB_FULL = FCORE // BF    # 16 blocks per core
KP = 114                 # padded pair-stack height: 50 + 14 zeros + 50
NSLOT = 4                # xt/xn buffer depth

F16 = mybir.dt.float16
F32 = mybir.dt.float32

XTH = BF * L // 2        # xt column split: 3200 elems (6400B) per half
XNH = NPAIR * D // 2     # xn column split: 4096 elems (8KB) per half


def build_program(n_blocks: int = NB_FULL) -> bass.Bass:
    nc = bass.Bass()
    nb = n_blocks

    xt_d = nc.declare_dram_parameter("xt", [nb, D, BF * L], F16, isOutput=False)
    xn_d = nc.declare_dram_parameter("xn", [nb, 2 * L, NPAIR * D], F16, isOutput=False)
    zr_d = nc.declare_dram_parameter("zeros14", [14, NPAIR * D], F16, isOutput=False)
    st_d = nc.declare_dram_parameter("st16", [D, nb * 16 * 16], F16, isOutput=False)
    id_d = nc.declare_dram_parameter("ident", [D, D], F16, isOutput=False)
    out_d = nc.declare_dram_parameter("pooledT", [nb, D, BF], F32, isOutput=True)

    with ExitStack() as ctx:
        e = ctx.enter_context
        xt_sb = [e(nc.sbuf_tensor(f"xt{i}", [D, BF * L], F16)) for i in range(NSLOT)]
        xn_sb = [e(nc.sbuf_tensor(f"xn{i}", [KP, NPAIR * D], F16)) for i in range(NSLOT)]
        st_sb = e(nc.sbuf_tensor("st_sb", [D, nb * 256], F16))
        id_sb = e(nc.sbuf_tensor("id_sb", [D, D], F16))
        stage_sb = [e(nc.sbuf_tensor(f"stage{i}", [16, 8 * L], F32)) for i in range(2)]
        scores_sb = [e(nc.sbuf_tensor(f"scores{i}", [BF, L], F32)) for i in range(2)]
        wexp_sb = [e(nc.sbuf_tensor(f"wexp{i}", [BF, L], F32)) for i in range(2)]
        den_sb = [e(nc.sbuf_tensor(f"den{i}", [BF, 1], F32)) for i in range(2)]
        rden_sb = [e(nc.sbuf_tensor(f"rden{i}", [BF, 1], F32)) for i in range(2)]
        wm_sb = [e(nc.sbuf_tensor(f"wm{i}", [BF, L], F16)) for i in range(2)]
        wmbd_sb = [e(nc.sbuf_tensor(f"wmbd{i}", [KP, BF], F16)) for i in range(2)]
        pooled_sb = [e(nc.sbuf_tensor(f"pooled{i}", [D, BF], F32)) for i in range(2)]
        # PSUM: one full 2KB bank per tensor so no two tensors share a bank
        ps1 = [e(nc.psum_tensor(f"ps1_{i}", [16, 512], F32)) for i in range(2)]
        pst = [e(nc.psum_tensor(f"pst{i}", [L, 1024], F16)) for i in range(2)]
        ps2 = [e(nc.psum_tensor(f"ps2_{i}", [D, 512], F32)) for i in range(2)]

        s_c1 = e(nc.semaphore("s_c1"))  # st16 loaded
        s_c2 = e(nc.semaphore("s_c2"))  # ident loaded
        s_z = e(nc.semaphore("s_z"))
        # per-slot load-completion sems, split by ring (A=qSP, B=qAct)
        s_ldtA = [e(nc.semaphore(f"s_ldtA{i}")) for i in range(NSLOT)]
        s_ldtB = [e(nc.semaphore(f"s_ldtB{i}")) for i in range(NSLOT)]
        s_ldnA = [e(nc.semaphore(f"s_ldnA{i}")) for i in range(NSLOT)]
        s_ldnB = [e(nc.semaphore(f"s_ldnB{i}")) for i in range(NSLOT)]
        s_zn = [e(nc.semaphore(f"s_zn{i}")) for i in range(NSLOT)]
        s_mm1 = e(nc.semaphore("s_mm1"))
        s_st = e(nc.semaphore("s_st"))
        s_sc = e(nc.semaphore("s_sc"))
        s_exp = e(nc.semaphore("s_exp"))
        s_sm = e(nc.semaphore("s_sm"))
        s_tr = e(nc.semaphore("s_tr"))
        s_bd = e(nc.semaphore("s_bd"))
        s_mm2 = e(nc.semaphore("s_mm2"))
        s_pc = e(nc.semaphore("s_pc"))
        s_od = e(nc.semaphore("s_od"))
        s_rc = e(nc.semaphore("s_rc"))

        with nc.Block() as block:

            @block.sync
            def _(sync):
                for b in range(nb):
                    if b >= NSLOT:  # slot reuse: block b-NSLOT's readers done
                        sync.wait_ge(s_mm1, b - NSLOT + 1)
                        sync.wait_ge(s_mm2, b - NSLOT + 1)
                    s = b % NSLOT
                    sync.dma_start(
                        xt_sb[s][:, 0:XTH], xt_d[b, :, 0:XTH]
                    ).then_inc(s_ldtA[s], 16)
                    sync.dma_start(
                        xn_sb[s][0:L, 0:XNH], xn_d[b, 0:L, 0:XNH]
                    ).then_inc(s_ldnA[s], 16)
                    sync.dma_start(
                        xn_sb[s][0:L, XNH:], xn_d[b, 0:L, XNH:]
                    ).then_inc(s_ldnA[s], 16)

            @block.scalar
            def _(scalar):
                # qAct-ring prologue: pad-zeroing, one-hot self vectors, identity
                scalar.dma_start(st_sb[:], st_d[:]).then_inc(s_c1, 16)
                scalar.dma_start(id_sb[:], id_d[:]).then_inc(s_c2, 16)
                for i in range(NSLOT):
                    scalar.dma_start(xn_sb[i][L:64, :], zr_d[:]).then_inc(s_zn[i], 16)
                # prefetch ring-B halves for the first blocks
                for b in range(min(3, nb)):
                    s = b % NSLOT
                    scalar.dma_start(
                        xt_sb[s][:, XTH:], xt_d[b, :, XTH:]
                    ).then_inc(s_ldtB[s], 16)
                    scalar.dma_start(
                        xn_sb[s][64 : 64 + L, 0:XNH], xn_d[b, L : 2 * L, 0:XNH]
                    ).then_inc(s_ldnB[s], 16)
                    scalar.dma_start(
                        xn_sb[s][64 : 64 + L, XNH:], xn_d[b, L : 2 * L, XNH:]
                    ).then_inc(s_ldnB[s], 16)
                for b in range(nb):
                    # stage copy: psum [16, 400] -> sbuf
                    scalar.wait_ge(s_mm1, b + 1)
                    if b >= 2:
                        scalar.wait_ge(s_sc, 16 * (b - 1))  # stage slot free
                    nc.scalar.copy(
                        stage_sb[b % 2][:], ps1[b % 2][:, 0 : 8 * L]
                    ).then_inc(s_st, 1)
                    # exp + accumulate denominator
                    scalar.wait_ge(s_sc, 16 * (b + 1))
                    if b >= 2:
                        scalar.wait_ge(s_sm, b - 1)  # wexp/den slot free
                    nc.scalar.activation(
                        wexp_sb[b % 2][:],
                        scores_sb[b % 2][:],
                        mybir.ActivationFunctionType.Exp,
                        accum_out=den_sb[b % 2][:],
                    ).then_inc(s_exp, 1)
                    # issue ring-B half of block b+3's loads
                    p = b + 3
                    if p < nb:
                        s = p % NSLOT
                        if b >= 1:  # slot occupant p-NSLOT = b-1: readers done
                            scalar.wait_ge(s_mm1, b)
                            scalar.wait_ge(s_mm2, b)
                        scalar.dma_start(
                            xt_sb[s][:, XTH:], xt_d[p, :, XTH:]
                        ).then_inc(s_ldtB[s], 16)
                        scalar.dma_start(
                            xn_sb[s][64 : 64 + L, 0:XNH], xn_d[p, L : 2 * L, 0:XNH]
                        ).then_inc(s_ldnB[s], 16)
                        scalar.dma_start(
                            xn_sb[s][64 : 64 + L, XNH:], xn_d[p, L : 2 * L, XNH:]
                        ).then_inc(s_ldnB[s], 16)

            @block.tensor
            def _(tensor):
                tensor.wait_ge(s_c1, 16)
                tensor.wait_ge(s_c2, 16)
                for b in range(nb + 1):
                    if b < nb:
                        # ---- phase 1 (block b): 16 chunk matmuls ----
                        g = b // NSLOT + 1
                        tensor.wait_ge(s_ldtA[b % NSLOT], 16 * g)
                        tensor.wait_ge(s_ldtB[b % NSLOT], 16 * g)
                        if b >= 2:
                            tensor.wait_ge(s_st, b - 1)  # ps1 slot free
                        for jj in range(16):
                            f0 = jj * 8
                            mm = nc.tensor.matmul(
                                ps1[b % 2][:, 0 : 8 * L],
                                st_sb[:, (b * 16 + jj) * 16 : (b * 16 + jj) * 16 + 16],
                                xt_sb[b % NSLOT][:, f0 * L : f0 * L + 8 * L],
                                start=(jj == 0),
                                stop=(jj == 15),
                            )
                        mm.then_inc(s_mm1, 1)
                    if b >= 1:
                        c = b - 1
                        # ---- transpose wm(c) -> [L, BF] ----
                        tensor.wait_ge(s_sm, c + 1)
                        if c >= 2:
                            tensor.wait_ge(s_bd, c - 1)  # pst slot free
                        nc.tensor.transpose(
                            pst[c % 2][:, 0:BF], wm_sb[c % 2][:], id_sb[:]
                        ).then_inc(s_tr, 1)
                        # ---- phase 2 (block c): 64 pair matmuls ----
                        gc = c // NSLOT + 1
                        tensor.wait_ge(s_ldnA[c % NSLOT], 32 * gc)
                        tensor.wait_ge(s_ldnB[c % NSLOT], 32 * gc)
                        if c < NSLOT:
                            tensor.wait_ge(s_zn[c % NSLOT], 16)
                        tensor.wait_ge(s_bd, c + 1)
                        if c >= 2:
                            tensor.wait_ge(s_pc, c - 1)  # ps2 slot free
                        for p in range(NPAIR):
                            mm = nc.tensor.matmul(
                                ps2[c % 2][:, 2 * p : 2 * p + 2],
                                xn_sb[c % NSLOT][:, p * D : (p + 1) * D],
                                wmbd_sb[c % 2][:, 2 * p : 2 * p + 2],
                                start=True,
                                stop=True,
                            )
                        mm.then_inc(s_mm2, 1)

            @block.vector
            def _(vector):
                for b in range(nb):
                    # softmax normalize -> wm (f16)
                    vector.wait_ge(s_exp, b + 1)
                    nc.vector.reciprocal(rden_sb[b % 2][:], den_sb[b % 2][:]).then_inc(
                        s_rc, 1
                    )
                    vector.wait_ge(s_rc, b + 1)  # same-engine RAW (deep pipe)
                    if b >= 2:
                        vector.wait_ge(s_tr, b - 1)  # wm slot free
                    nc.vector.tensor_scalar_mul(
                        wm_sb[b % 2][:], wexp_sb[b % 2][:], rden_sb[b % 2][:]
                    ).then_inc(s_sm, 1)
                    # block-diag columns from transposed weights
                    vector.wait_ge(s_tr, b + 1)
                    if b == 0:
                        vector.wait_ge(s_z, 2)  # wmbd zero-init done
                    if b >= 2:
                        vector.wait_ge(s_mm2, b - 1)  # wmbd slot free
                    pt3 = pst[b % 2][:, 0:BF].rearrange("p (pr two) -> p pr two", two=2)
                    lo = wmbd_sb[b % 2][0:L, :].rearrange("p (pr two) -> p pr two", two=2)
                    hi = wmbd_sb[b % 2][64 : 64 + L, :].rearrange(
                        "p (pr two) -> p pr two", two=2
                    )
                    nc.vector.tensor_copy(lo[:, :, 0:1], pt3[:, :, 0:1])
                    nc.vector.tensor_copy(hi[:, :, 1:2], pt3[:, :, 1:2]).then_inc(
                        s_bd, 1
                    )
                    # evacuate pooled^T
                    vector.wait_ge(s_mm2, b + 1)
                    if b >= 2:
                        vector.wait_ge(s_od, 16 * (b - 1))  # pooled slot free
                    nc.vector.tensor_copy(
                        pooled_sb[b % 2][:], ps2[b % 2][:, 0:BF]
                    ).then_inc(s_pc, 1)

            @block.gpsimd
            def _(gpsimd):
                # one-time zero-init of both wmbd slots (off-diagonal zeros +
                # pad rows persist; per-block copies only overwrite diagonals)
                nc.gpsimd.memset(wmbd_sb[0][:], 0.0).then_inc(s_z, 1)
                nc.gpsimd.memset(wmbd_sb[1][:], 0.0).then_inc(s_z, 1)
                for b in range(nb):
                    # scatter scores: [16 slots, 8 friends, 50] -> [128, 50]
                    gpsimd.wait_ge(s_st, b + 1)
                    if b >= 1:
                        gpsimd.wait_ge(s_sc, 16 * b)  # own-sem update order
                    if b >= 2:
                        gpsimd.wait_ge(s_exp, b - 1)  # scores slot free
                    gpsimd.dma_start(
                        scores_sb[b % 2][:],
                        stage_sb[b % 2][:].rearrange("s (f l) -> s f l", l=L),
                    ).then_inc(s_sc, 16)
                    # output DMA (previous block, to keep scatter ahead)
                    if b >= 1:
                        gpsimd.wait_ge(s_pc, b)
                        gpsimd.wait_ge(s_od, 16 * (b - 1))  # own-sem update order
                        gpsimd.dma_start(
                            out_d[b - 1], pooled_sb[(b - 1) % 2][:]
                        ).then_inc(s_od, 16)
                gpsimd.wait_ge(s_pc, nb)
                gpsimd.wait_ge(s_od, 16 * (nb - 1))
                gpsimd.dma_start(out_d[nb - 1], pooled_sb[(nb - 1) % 2][:]).then_inc(
                    s_od, 16
                )
                gpsimd.wait_ge(s_od, 16 * nb)

    nc.finalize()
    return nc


def pack_inputs(friend_diff_x, self_x, friend_diff_src_mask,
                n_blocks: int = NB_FULL, ncores: int = NCORES):
    """Host-side fp16 packing + per-core slicing. Returns list of in_maps."""
    x16 = np.asarray(friend_diff_x, dtype=np.float32).astype(np.float16)
    nblk_total = ncores * n_blocks
    nrows = nblk_total * BF
    x16 = x16[:nrows]
    mk = np.asarray(friend_diff_src_mask)[:nrows, :, None]  # [rows, L, 1] bool
    # xt: [blk, d, f, l]  (unmasked)
    xt = np.ascontiguousarray(
        x16.reshape(nblk_total, BF, L, D).transpose(0, 3, 1, 2)
    ).reshape(nblk_total, D, BF * L)
    # xn: mask-premultiplied, [blk, fip, l, pair, d] padded to 114 rows:
    # rows 0:50 = fip0, rows 50:64 = zeros, rows 64:114 = fip1
    xm = np.where(mk, x16, np.float16(0))
    xm5 = xm.reshape(nblk_total, NPAIR, 2, L, D).transpose(0, 2, 3, 1, 4)
    xn = np.ascontiguousarray(xm5).reshape(nblk_total, 2 * L, NPAIR * D)
    # st16: [d, blk, jj, m] = s_{2*blk + jj//8}[d] if m == jj else 0
    s16 = np.asarray(self_x, dtype=np.float32).astype(np.float16)  # [B, D]
    n_users_total = 2 * nblk_total
    st16 = np.zeros((D, nblk_total, 16, 16), dtype=np.float16)
    for jj in range(16):
        st16[:, :, jj, jj] = s16[:n_users_total].reshape(nblk_total, 2, D)[
            :, jj // 8, :
        ].T
    st16 = st16.reshape(D, nblk_total * 256)
    ident = np.eye(D, dtype=np.float16)

    in_maps = []
    for i in range(ncores):
        in_maps.append(
            {
                "xt": xt[i * n_blocks : (i + 1) * n_blocks],
                "xn": xn[i * n_blocks : (i + 1) * n_blocks],
                "zeros14": np.zeros((14, NPAIR * D), dtype=np.float16),
                "st16": np.ascontiguousarray(
                    st16[:, i * n_blocks * 256 : (i + 1) * n_blocks * 256]
                ),
                "ident": ident,
            }
        )
    return in_maps


def unpack_output(pooledT_list, n_blocks: int = NB_FULL):
    """[ncores][n_blocks, D, BF] f32 -> [rows, D]"""
    full = np.stack(pooledT_list)  # [ncores, nb, D, BF]
    return full.transpose(0, 1, 3, 2).reshape(-1, D)


_NC_CACHE = {}


def kernel(friend_diff_x, self_x, friend_num_src, friend_num_src_tensor,
           friend_diff_src_mask, _trace=False, _trace_kwargs=None):
    assert int(friend_num_src) == FPER
    if "nc" not in _NC_CACHE:
        _NC_CACHE["nc"] = build_program(NB_FULL)
    nc = _NC_CACHE["nc"]
    in_maps = pack_inputs(friend_diff_x, self_x, friend_diff_src_mask)
    kw = {}
    if _trace:
        kw = dict(trace=True, trace_kwargs=_trace_kwargs or {})
    res = run_bass_kernel_spmd(nc, in_maps, list(range(NCORES)), **kw)
    out = unpack_output([res.results[i]["pooledT"] for i in range(NCORES)])
    kernel._last_results = res
    return out.reshape(B, FPER, D).astype(np.float32)
